# revision 24
# baseline (speedup 1.0000x reference)
"""Trainium2 Bass kernel for MultiHeadSelfAttention (cross-attention variant).

Problem: B=2, LQ=LK=2048, D=1024, H=16, d_k=64, fp32.
  q_a = cdd @ W_q + b_q ; k_a = his @ W_k + b_k ; v_a = his @ W_v + b_v
  S = q k^T / 8 ; A = exp(S) / (sum_k exp(S) + 1e-8) ; ctx = A v
  returns (context, q_a)

Sharding (8 cores, no collectives): core c handles batch c//4 and head-block
c%4 (4 heads = 256 columns of W_q/W_k/W_v).  Each core writes disjoint column
slices of both outputs; the host gathers them.

Per-core dataflow (all matmuls in float32r = full-rate fp32 mode):
  - PE-transpose cdd[b]/his[b] chunks into feature-major layout
  - Q^T/K^T projections accumulate in PSUM over 8 feature tiles (W stationary)
  - V projection computed in natural token-major layout (X^T stationary)
  - attention per (q-chunk 512, head): S^T tiles = K_tile @ Q^T (PSUM), ACT
    exp(x*0.125) -> SBUF, ctx^T += [V|1]^T @ expS^T (ones column gives the
    row-sum for free), then PE-transpose [65,128] blocks and normalize with
    DVE reciprocal (per-partition scalar).
  - attention is software-pipelined (MM2 of group g emitted after the score
    matmuls + exp of group g+1) so the PE never waits on the ACT exp; the
    Q-side projection work of chunk ch+1 drips into attention(ch)'s PE idle
    slots ("fine" order).  Measured ~147 us/core on HW, at the ACT exp floor.
"""

import numpy as np
from contextlib import ExitStack

B = 2
L = 2048
D = 1024
H = 16
DK = 64
P = 128
NCORES = 8
CPB = 4  # cores per batch
HPC = H // CPB  # heads per core = 4
COLS = HPC * DK  # 256 output columns per core
CHUNK = 512  # token chunk (max fp32 moving operand)

_CACHE = {}


def _build(L=L, D=D, COLS=COLS, cfg=None, repeat=1):
    import concourse.tile as tile
    from concourse import bacc, masks, mybir

    f32 = mybir.dt.float32
    f32r = mybir.dt.float32r
    Exp = mybir.ActivationFunctionType.Exp
    add_op = mybir.AluOpType.add

    HL = COLS // DK  # heads handled locally
    FT = D // P  # feature tiles
    TT = L // P  # token tiles
    TCH = L // CHUNK  # token chunks
    CT = COLS // P  # column tiles
    IT = CHUNK // P  # token tiles per chunk = 4
    VW = DK + 1  # 65: V columns + ones column

    cfg = dict(
        dict(tp=2, pp=1, sp=2, cp=1, es=3, gk=2, order="fine", dvet=False, swp=True),
        **(cfg or {}),
    )

    nc = bacc.Bacc(
        "TRN2",
        target_bir_lowering=False,
        debug=False,
        num_devices=NCORES,
    )

    x_q = nc.dram_tensor("x_q", [L, D], f32, kind="ExternalInput").ap()
    x_kv = nc.dram_tensor("x_kv", [L, D], f32, kind="ExternalInput").ap()
    w_q = nc.dram_tensor("w_q", [D, COLS], f32, kind="ExternalInput").ap()
    w_k = nc.dram_tensor("w_k", [D, COLS], f32, kind="ExternalInput").ap()
    w_v = nc.dram_tensor("w_v", [D, COLS], f32, kind="ExternalInput").ap()
    b_q = nc.dram_tensor("b_q", [COLS], f32, kind="ExternalInput").ap()
    b_k = nc.dram_tensor("b_k", [COLS], f32, kind="ExternalInput").ap()
    b_v = nc.dram_tensor("b_v", [COLS], f32, kind="ExternalInput").ap()
    q_out = nc.dram_tensor("q_out", [L, COLS], f32, kind="ExternalOutput").ap()
    c_out = nc.dram_tensor("c_out", [L, COLS], f32, kind="ExternalOutput").ap()

    with tile.TileContext(nc) as tc, ExitStack() as ctx:
        singles = ctx.enter_context(tc.tile_pool(name="singles", bufs=1))

        identity = singles.tile([P, P], f32)
        masks.make_identity(nc, identity[:])

        # biases: q/k as per-partition scalars in ^T layout; v broadcast to rows
        bq_sb = singles.tile([P, CT], f32)
        bk_sb = singles.tile([P, CT], f32)
        nc.sync.dma_start(bq_sb[:], b_q.rearrange("(c p) -> p c", p=P))
        nc.sync.dma_start(bk_sb[:], b_k.rearrange("(c p) -> p c", p=P))
        bv_row = singles.tile([1, COLS], f32)
        nc.sync.dma_start(bv_row[:], b_v.rearrange("(o c) -> o c", o=1))
        bv_bcast = singles.tile([P, COLS], f32)
        nc.gpsimd.partition_broadcast(bv_bcast[:], bv_row[:1])

        # weights: [D, COLS] -> [128, FT, COLS], rounded to f32r via DVE
        wq_sb = singles.tile([P, FT * COLS], f32r)
        wk_sb = singles.tile([P, FT * COLS], f32r)
        wv_sb = singles.tile([P, FT * COLS], f32r)
        wq_sb = wq_sb.rearrange("p (f c) -> p f c", f=FT)
        wk_sb = wk_sb.rearrange("p (f c) -> p f c", f=FT)
        wv_sb = wv_sb.rearrange("p (f c) -> p f c", f=FT)
        wstage_pool = ctx.enter_context(tc.tile_pool(name="wstage", bufs=1))
        for wsb, wdr in ((wq_sb, w_q), (wk_sb, w_k), (wv_sb, w_v)):
            wst = wstage_pool.tile([P, FT * COLS], f32, tag="wst")
            wst = wst.rearrange("p (f c) -> p f c", f=FT)
            nc.sync.dma_start(wst[:], wdr.rearrange("(f p) c -> p f c", p=P))
            nc.vector.tensor_copy(wsb[:], wst[:])

        # persistent attention operands
        QT = singles.tile([P, CT * L], f32r)
        KT = singles.tile([P, CT * L], f32r)
        V = singles.tile([P, TT * HL * VW], f32r)
        QT = QT.rearrange("p (c l) -> p c l", c=CT)
        KT = KT.rearrange("p (c l) -> p c l", c=CT)
        V = V.rearrange("p (t h w) -> p t h w", t=TT, h=HL)
        ones1 = singles.tile([P, 1], f32)
        nc.vector.memset(ones1[:], 1.0)
        nc.vector.tensor_copy(
            V[:, :, :, DK : DK + 1], ones1[:].to_broadcast((P, TT, HL, 1))
        )

        # ---- pools (PSUM total = 8 banks: tp 2 + pp 1 + sp 4 + cp 1) ----
        xnat_pool = ctx.enter_context(tc.tile_pool(name="xnat", bufs=cfg.get("xnat", 4)))
        qnat_pool = ctx.enter_context(tc.tile_pool(name="qnat", bufs=3))
        xt_pool = ctx.enter_context(tc.tile_pool(name="xt", bufs=2))
        tpsum = ctx.enter_context(tc.tile_pool(name="tpsum", bufs=cfg["tp"], space="PSUM"))
        ppsum = ctx.enter_context(tc.tile_pool(name="ppsum", bufs=cfg["pp"], space="PSUM"))
        spool = ctx.enter_context(tc.tile_pool(name="spool", bufs=cfg["sp"], space="PSUM"))
        cpool = ctx.enter_context(tc.tile_pool(name="cpool", bufs=cfg["cp"], space="PSUM"))
        espool = ctx.enter_context(tc.tile_pool(name="es", bufs=cfg["es"]))
        ctpool = ctx.enter_context(tc.tile_pool(name="ct", bufs=2))
        ctxpool = ctx.enter_context(tc.tile_pool(name="ctxsb", bufs=cfg.get("ctxb", 2)))
        recpool = ctx.enter_context(tc.tile_pool(name="rec", bufs=4))

        GK = cfg["gk"]  # score k-tiles per PSUM group / exp instruction

        def lt_load(xdram, tag, ch):
            tok0 = ch * CHUNK
            xnats = []
            for it in range(IT):
                xn = xnat_pool.tile([P, D], f32, tag=f"xn{tag}", name=f"xn{tag}")
                nc.sync.dma_start(
                    xn[:], xdram[tok0 + it * P : tok0 + (it + 1) * P, :]
                )
                xnats.append(xn)
            xt = xt_pool.tile([P, FT * CHUNK], f32r, tag=f"xt{tag}", name=f"xt{tag}")
            xt = xt.rearrange("p (f l) -> p f l", f=FT)
            return xnats, xt

        def lt_transpose(xnats, xt, ft, drain):
            tp = tpsum.tile([P, CHUNK], f32, tag="tpsum", name="tp")
            for it in range(IT):
                nc.tensor.transpose(
                    tp[:, it * P : (it + 1) * P],
                    xnats[it][:, ft * P : (ft + 1) * P],
                    identity[:],
                )
            drain(xt[:, ft, :], tp[:])

        def load_transpose(xdram, tag, ch, drain=None):
            """DMA a 512-token chunk and PE-transpose to feature-major f32r."""
            if cfg["dvet"] is True or (cfg["dvet"] == "q" and tag == "q"):
                return load_transpose_dve(xdram, tag, ch)
            drain = drain or nc.scalar.copy
            xnats, xt = lt_load(xdram, tag, ch)
            for ft in range(FT):
                lt_transpose(xnats, xt, ft, drain)
            return xt

        def load_transpose_dve(xdram, tag, ch):
            """Block-swapped DMA load + DVE StreamTranspose (no PE work).

            dst S[32A+v, 32B+u] = X[tok0+32B+v, 128ft+32A+u]; stream-transpose
            of 32x32 blocks then yields X^T exactly.
            """
            tok0 = ch * CHUNK
            xt = xt_pool.tile([P, FT * CHUNK], f32r, tag=f"xt{tag}", name=f"xt{tag}")
            xt = xt.rearrange("p (f l) -> p f l", f=FT)
            for ft in range(FT):
                s = xnat_pool.tile([P, CHUNK], f32, tag=f"xs{tag}", name=f"xs{tag}")
                blk = xdram[tok0 : tok0 + CHUNK, ft * P : (ft + 1) * P]
                swz = blk.rearrange("(b v) (a u) -> a v b u", v=32, u=32)
                dst = s.rearrange("p (b u) -> p b u", u=32)
                for a in range(4):
                    nc.sync.dma_start(dst[a * 32 : (a + 1) * 32], swz[a])
                nc.vector.transpose(xt[:, ft, :], s[:])
            return xt

        def proj_T_ct(wsb, xt, bsb, OUT, ch, ct):
            tok0 = ch * CHUNK
            pp = ppsum.tile([P, CHUNK], f32, tag="ppsum", name="pp")
            for ft in range(FT):
                nc.tensor.matmul(
                    pp[:],
                    wsb[:, ft, ct * P : (ct + 1) * P],
                    xt[:, ft, :],
                    start=(ft == 0),
                    stop=(ft == FT - 1),
                )
            nc.vector.tensor_scalar_add(
                OUT[:, ct, tok0 : tok0 + CHUNK], pp[:], bsb[:, ct : ct + 1]
            )

        def proj_T(wsb, xt, bsb, OUT, ch):
            """^T-layout projection (columns on partitions) with bias."""
            for ct in range(CT):
                proj_T_ct(wsb, xt, bsb, OUT, ch, ct)

        def proj_v(xt_kv, ch):
            """natural-layout V projection (tokens on partitions) with bias."""
            for it in range(IT):
                pv = ppsum.tile([P, COLS], f32, tag="ppsum", name="pv")
                for ft in range(FT):
                    nc.tensor.matmul(
                        pv[:],
                        xt_kv[:, ft, it * P : (it + 1) * P],
                        wv_sb[:, ft, :],
                        start=(ft == 0),
                        stop=(ft == FT - 1),
                    )
                nc.vector.tensor_tensor(
                    V[:, ch * IT + it, :, 0:DK],
                    pv[:].rearrange("p (h w) -> p h w", h=HL),
                    bv_bcast[:].rearrange("p (h w) -> p h w", h=HL),
                    op=add_op,
                )

        def qa_out_one(ch, ct, it):
            tok0 = ch * CHUNK
            tq = tpsum.tile([P, CHUNK], f32, tag="tpsum", name="tq")
            nc.tensor.transpose(
                tq[:, :P],
                QT[:, ct, tok0 + it * P : tok0 + (it + 1) * P].bitcast(f32),
                identity[:],
            )
            qn = qnat_pool.tile([P, P], f32, tag="qn", name="qn")
            nc.vector.tensor_copy(qn[:], tq[:, :P])
            nc.sync.dma_start(
                q_out[tok0 + it * P : tok0 + (it + 1) * P, ct * P : (ct + 1) * P],
                qn[:],
            )

        def qa_out(ch):
            """transpose Q^T chunk back to natural layout and DMA out."""
            for ct in range(CT):
                for it in range(IT):
                    qa_out_one(ch, ct, it)

        def attention(qc, feed=None):
            q0 = qc * CHUNK
            ctx_tiles = [
                ctxpool.tile([P, COLS], f32, tag=f"ctx{it}", name=f"ctxt{it}")
                for it in range(IT)
            ]
            for h in range(HL):
                ct, hh = divmod(h, HL // CT)
                rows = slice(hh * DK, (hh + 1) * DK)
                cpsum = cpool.tile([P, CHUNK], f32, tag="cpsum", name="cpsum")

                def mm2(g, es):
                    for j in range(GK):
                        kt = g * GK + j
                        nc.tensor.matmul(
                            cpsum[:VW, :],
                            V[:, kt, h, :],
                            es[:, j * CHUNK : (j + 1) * CHUNK],
                            start=(kt == 0),
                            stop=(kt == TT - 1),
                        )

                prev = None
                for g in range(TT // GK):
                    sp = spool.tile([P, GK * CHUNK], f32, tag="sp", name="sp")
                    for j in range(GK):
                        kt = g * GK + j
                        nc.tensor.matmul(
                            sp[:, j * CHUNK : (j + 1) * CHUNK],
                            KT[rows, ct, kt * P : (kt + 1) * P],
                            QT[rows, ct, q0 : q0 + CHUNK],
                            start=True,
                            stop=True,
                        )
                    es = espool.tile([P, GK * CHUNK], f32r, tag="es", name="es")
                    nc.scalar.activation(es[:], sp[:], Exp, scale=0.125)
                    if cfg["swp"]:
                        if prev is not None:
                            mm2(*prev)
                        prev = (g, es)
                    else:
                        mm2(g, es)
                    if feed is not None:
                        feed()
                if cfg["swp"] and prev is not None:
                    mm2(*prev)
                # normalize: transpose [65,128] blocks, reciprocal, scale
                ctT = ctpool.tile([P, CHUNK], f32, tag="ctT", name="ctT")
                nc.vector.tensor_copy(ctT[:VW, :], cpsum[:VW, :])
                for it in range(IT):
                    t2 = tpsum.tile([P, CHUNK], f32, tag="tpsum", name="t2")
                    nc.tensor.transpose(
                        t2[:, :VW],
                        ctT[:VW, it * P : (it + 1) * P],
                        identity[:VW, :VW],
                    )
                    rec = recpool.tile([P, 2], f32, tag="rec", name="rec")
                    nc.vector.tensor_scalar_add(
                        rec[:, 0:1], t2[:, DK : DK + 1], 1e-8
                    )
                    nc.vector.reciprocal(rec[:, 1:2], rec[:, 0:1])
                    nc.vector.tensor_scalar_mul(
                        ctx_tiles[it][:, h * DK : (h + 1) * DK],
                        t2[:, 0:DK],
                        rec[:, 1:2],
                    )
            for it in range(IT):
                nc.sync.dma_start(
                    c_out[q0 + it * P : q0 + (it + 1) * P, :], ctx_tiles[it][:]
                )

        def emit_all():
            # K/V first (they gate every attention chunk), then Q chunks with
            # attention interleaved so exp starts as early as possible.
            if cfg["order"] == "interleave":
                for ch in range(TCH):
                    xt_kv = load_transpose(x_kv, "kv", ch)
                    proj_T(wk_sb, xt_kv, bk_sb, KT, ch)
                    proj_v(xt_kv, ch)
                for qc in range(TCH):
                    xt_q = load_transpose(x_q, "q", qc)
                    proj_T(wq_sb, xt_q, bq_sb, QT, qc)
                    qa_out(qc)
                    attention(qc)
            elif cfg["order"] == "qfirst":
                for ch in range(TCH):
                    xt_kv = load_transpose(x_kv, "kv", ch)
                    proj_T(wk_sb, xt_kv, bk_sb, KT, ch)
                    proj_v(xt_kv, ch)
                for qc in range(TCH):
                    xt_q = load_transpose(x_q, "q", qc)
                    proj_T(wq_sb, xt_q, bq_sb, QT, qc)
                    qa_out(qc)
                for qc in range(TCH):
                    attention(qc)
            elif cfg["order"] == "fine":
                # K/V fully pre-attention (every attention group needs them);
                # Q-side work for chunk ch+1 drips into attention(qc=ch)'s PE
                # idle slots (ACT-bound there).
                for ch in range(TCH):
                    xt_kv = load_transpose(x_kv, "kv", ch, drain=nc.scalar.copy)
                    proj_T(wk_sb, xt_kv, bk_sb, KT, ch)
                    proj_v(xt_kv, ch)
                units = []

                def q_side_units(ch):
                    if cfg["dvet"] == "q" or cfg["dvet"] is True:
                        xt = load_transpose_dve(x_q, "q", ch)
                    else:
                        xnats, xt = lt_load(x_q, "q", ch)
                        for ft in range(FT):
                            units.append(
                                lambda xn=xnats, x=xt, f=ft: lt_transpose(
                                    xn, x, f, nc.vector.tensor_copy
                                )
                            )
                    for ct in range(CT):
                        units.append(
                            lambda x=xt, c=ct, k=ch: proj_T_ct(
                                wq_sb, x, bq_sb, QT, k, c
                            )
                        )
                    for ct in range(CT):
                        for it in range(IT):
                            units.append(lambda k=ch, c=ct, i=it: qa_out_one(k, c, i))

                def feed():
                    if units:
                        units.pop(0)()

                # chunk 0 Q-side eagerly (gates first attention chunk)
                xt_q0 = load_transpose(x_q, "q", 0, drain=nc.scalar.copy)
                proj_T(wq_sb, xt_q0, bq_sb, QT, 0)
                qa_out(0)
                for qc in range(TCH):
                    if qc + 1 < TCH:
                        q_side_units(qc + 1)
                    attention(qc, feed=feed)
                    while units:  # safety: drain leftovers before next chunk
                        units.pop(0)()
            else:  # phased: original structure
                for ch in range(TCH):
                    xt_q = load_transpose(x_q, "q", ch)
                    xt_kv = load_transpose(x_kv, "kv", ch)
                    proj_T(wq_sb, xt_q, bq_sb, QT, ch)
                    proj_T(wk_sb, xt_kv, bk_sb, KT, ch)
                    proj_v(xt_kv, ch)
                    qa_out(ch)
                for qc in range(TCH):
                    attention(qc)


        for _rep in range(repeat):
            emit_all()
    nc.compile()
    return nc


def _get_nc():
    if "nc" not in _CACHE:
        _CACHE["nc"] = _build()
    return _CACHE["nc"]


def make_in_maps(cdd, his, W_q, b_q, W_k, b_k, W_v, b_v):
    cdd = np.asarray(cdd, dtype=np.float32)
    his = np.asarray(his, dtype=np.float32)
    W_q = np.asarray(W_q, dtype=np.float32)
    W_k = np.asarray(W_k, dtype=np.float32)
    W_v = np.asarray(W_v, dtype=np.float32)
    b_q = np.asarray(b_q, dtype=np.float32)
    b_k = np.asarray(b_k, dtype=np.float32)
    b_v = np.asarray(b_v, dtype=np.float32)
    in_maps = []
    for c in range(NCORES):
        b, hb = divmod(c, CPB)
        sl = slice(hb * COLS, (hb + 1) * COLS)
        in_maps.append(
            {
                "x_q": np.ascontiguousarray(cdd[b]),
                "x_kv": np.ascontiguousarray(his[b]),
                "w_q": np.ascontiguousarray(W_q[:, sl]),
                "w_k": np.ascontiguousarray(W_k[:, sl]),
                "w_v": np.ascontiguousarray(W_v[:, sl]),
                "b_q": np.ascontiguousarray(b_q[sl]),
                "b_k": np.ascontiguousarray(b_k[sl]),
                "b_v": np.ascontiguousarray(b_v[sl]),
            }
        )
    return in_maps


def assemble_outputs(results):
    context = np.zeros((B, L, D), dtype=np.float32)
    q_a = np.zeros((B, L, D), dtype=np.float32)
    for c, out in enumerate(results):
        b, hb = divmod(c, CPB)
        sl = slice(hb * COLS, (hb + 1) * COLS)
        q_a[b, :, sl] = out["q_out"]
        context[b, :, sl] = out["c_out"]
    return (context, q_a)


def kernel(cdd, his, W_q, b_q, W_k, b_k, W_v, b_v):
    from concourse.bass_utils import run_bass_kernel_spmd

    nc = _get_nc()
    in_maps = make_in_maps(cdd, his, W_q, b_q, W_k, b_k, W_v, b_v)

    res = run_bass_kernel_spmd(
        nc, in_maps, core_ids=list(range(NCORES)), trace=_CACHE.get("trace", False)
    )
    _CACHE["last_result"] = res
    return assemble_outputs(res.results)

